# revision 2
# baseline (speedup 1.0000x reference)
"""Trainium2 Bass kernel for DigitConvolutionalModel (8-core data parallel).

Computation: x(B,784) -> 3x3 valid conv on 28x28 -> flatten(676)
             -> FC(100)+ReLU -> FC(10), B = 65536.

Algebraic restructure (host side, exact): the conv is linear, so conv and
fc1 fold into one 784->100 matrix W1eff (accumulated in float64). The
device kernel is then just two matmul layers per 512-sample tile:
  h = relu(x @ W1eff + b1);  y = h @ fc2_w.T + b2.

Numerics: the matmul datapath runs in fp16 (inputs rounded once on the
host). Measured end-to-end scale-relative absmax error vs the fp32
reference is ~4.5e-4; fp16 streams the PE at 1 col/cycle (fp32 runs at
~1/4 rate) and halves the HBM traffic, which is the kernel's bottleneck.

Per-core layout (B_shard=8192 = 16 tiles x 512):
  x is pre-transposed on the host to feature-major tiles so the matmul
  contraction (features) lands on SBUF partitions with no on-device
  transposes. Features 0..767 form 6 chunks of 128 partitions (full DMA
  port utilization, fully contiguous 0.77 MB loads alternating across
  the two HWDGE rings); the 16 remainder features for all 16 tiles are
  packed into one [128, 2048] tile at 32-aligned partition groups (PE
  row-group granularity) and applied with per-group w1r replicas.
  Outputs accumulate in SBUF and leave in tapered writebacks so only
  one 20 KB write remains after the final tile.
"""

import numpy as np

import concourse.bass as bass
import concourse.mybir as mybir
import concourse.tile as tile
from concourse.bass_utils import run_bass_kernel_spmd
from concourse.vector_clock import ScopedClock

N_CORES = 8
B_TOTAL = 65536
B_SHARD = B_TOTAL // N_CORES  # 8192
BT = 512  # batch tile (one PSUM bank of fp32)
N_TILES = B_SHARD // BT  # 16
FC = 6  # full 128-partition feature chunks (6*128 = 768)
F_REM = 784 - FC * 128  # 16 remainder features
H1 = 100
H2 = 10

_f32 = mybir.dt.float32
_f32r = mybir.dt.float32r
_f16 = mybir.dt.float16


class SplitDrainTileContext(tile.TileContext):
    """TileContext whose tail drain carries at most one sync wait.

    The pinned walrus rejects instructions with >2 sync waits
    ("Too many sync wait commands" in setupSyncWait); the stock tail
    drain accumulates one wait per active proc. Emit one drain per
    wait instead — consecutive drains on the sync engine are
    semantically equivalent to one drain carrying all the waits.
    """

    def _drain_and_barrier(self, tick_clock, wait_clock):
        nc = self.nc
        # Cheap tail: the stock version runs two full EVSEM butterflies
        # (~13us measured). Instead: gpsimd waits on the whole vector
        # clock (all tracked incs have landed), every engine drains its
        # own DGE queues, gpsimd clears the sem ranges, and one
        # sequencer-level sem-only barrier closes the kernel.
        drain_inst = nc.gpsimd.drain()
        wait_clock.add_sem_waits(
            drain_inst.ins, ScopedClock({None: tick_clock.global_clock})
        )
        raw = drain_inst.ins
        si = raw.sync_info
        if si is not None and si.on_wait and len(si.on_wait) > 1:
            waits = list(si.on_wait)
            si.on_wait = waits[:1]
            raw.sync_info = si
            for w in waits[1:]:
                extra = nc.gpsimd.drain()
                extra.ins.sync_info = mybir.SyncInfo(on_wait=[w], on_update=[])
        for eng in (nc.sync, nc.scalar, nc.vector, nc.tensor):
            eng.drain()

        # No tail barrier: gpsimd's global-clock waits above guarantee all
        # tracked sem incs (incl. DMA completions) have landed before the
        # clears, and NRT serializes re-executions on all-engine completion.
        assert self.sems is not None
        popped = nc._tile_sem_poison_stack.pop()
        assert popped is self._sem_poison
        nc.clear_and_free_semaphores(list(self.sems.allocated().values()))


def _split_sync_waits(nc: bass.Bass, limit: int = 1) -> None:
    """Walrus-compat post-pass: the pinned walrus rejects instructions
    carrying more than ~2 sync waits. Hoist excess waits onto NoOp
    instructions inserted just before the offending instruction on the
    same engine — semantically identical (waits run in stream order)."""
    n = 0
    for fn in nc.m.functions:
        for bb in fn.blocks:
            out = []
            changed = False
            for inst in bb.instructions:
                si = inst.sync_info
                if si is not None and si.on_wait and len(si.on_wait) > limit:
                    waits = list(si.on_wait)
                    for i in range(0, len(waits) - limit, limit):
                        nop = mybir.InstNoOp(
                            name=f"swsplit-{n}",
                            ins=[],
                            outs=[],
                            sync_info=mybir.SyncInfo(
                                on_wait=waits[i : i + limit], on_update=[]
                            ),
                        )
                        nop.engine = inst.engine
                        out.append(nop)
                        n += 1
                    si.on_wait = waits[len(waits) - limit :]
                    inst.sync_info = si
                    changed = True
                out.append(inst)
            if changed:
                bb.instructions = out


WARM_MM = 36  # bridges Tensor-ready (~7.6us) to first x bytes (~9.4us)


def _build_nc() -> bass.Bass:
    nc = bass.Bass(monotonic_sem_count=0)
    xm = nc.dram_tensor("xm", [N_TILES, 128, FC, BT], _f16, kind="ExternalInput")
    # remainder features packed 4 tile-groups x 16 features into 128
    # partitions at 32-aligned offsets (PE row-group granularity)
    xr = nc.dram_tensor("xr", [128, 4 * BT], _f16, kind="ExternalInput")
    w1m = nc.dram_tensor("w1m", [128, FC * H1], _f16, kind="ExternalInput")
    # w1r replicated at partition offsets 0/32/64/96
    w1r = nc.dram_tensor("w1r", [128, H1], _f16, kind="ExternalInput")
    b1 = nc.dram_tensor("b1", [H1, 1], _f32, kind="ExternalInput")
    w2 = nc.dram_tensor("w2", [H1, H2], _f16, kind="ExternalInput")
    b2 = nc.dram_tensor("b2", [H2, 1], _f32, kind="ExternalInput")
    y = nc.dram_tensor("y", [H2, N_TILES * BT], _f32, kind="ExternalOutput")

    with SplitDrainTileContext(nc) as tc:
        with (
            tc.tile_pool(name="consts", bufs=1) as cpool,
            tc.tile_pool(name="xp", bufs=N_TILES) as xpool,
            tc.tile_pool(name="hp", bufs=4) as hpool,
            tc.tile_pool(name="psh", bufs=4, space="PSUM") as psh,
            tc.tile_pool(name="pso", bufs=4, space="PSUM") as pso,
        ):
            w1m_sb = cpool.tile([128, FC * H1], _f16, tag="w1m")
            xr_sb = cpool.tile([128, 4 * BT], _f16, tag="xr")
            w1r_sb = cpool.tile([128, H1], _f16, tag="w1r")
            b1_sb = cpool.tile([H1, 1], _f32, tag="b1")
            w2_sb = cpool.tile([H1, H2], _f16, tag="w2")
            b2_sb = cpool.tile([H2, 1], _f32, tag="b2")
            # outputs accumulate here; written back in tapered chunks
            o_sb = cpool.tile([H2, N_TILES * BT], _f32, tag="o")
            warm_sb = cpool.tile([128, 64], _f16, tag="warm")

            # ---- all input DMAs issued upfront: both HWDGE rings stay
            # saturated end-to-end, no buffer-reuse waits (16 x bufs live).
            # Ring totals ~6.4/6.6 MB; ~210 GB/s each. First/last tiles
            # split in halves so the PE can start earlier / finish sooner.
            # xr split per 32-partition group g, delivered just before
            # quad g's remainder matmuls need it.
            x_sbs = [xpool.tile([128, FC * BT], _f16, tag="x", name="x") for _ in range(N_TILES)]
            hw = FC * BT // 2
            srcs = [xm[t].rearrange("p c b -> p (c b)") for t in range(N_TILES)]
            # sync ring
            nc.sync.dma_start(out=w1m_sb[:, :H1], in_=w1m[:, :H1])
            nc.sync.dma_start(out=x_sbs[0][:, :hw], in_=srcs[0][:, :hw])
            nc.sync.dma_start(out=x_sbs[0][:, hw:], in_=srcs[0][:, hw:])
            nc.sync.dma_start(out=x_sbs[2][:], in_=srcs[2])
            nc.sync.dma_start(out=xr_sb[32:64, :], in_=xr[32:64, :])
            nc.sync.dma_start(out=x_sbs[4][:], in_=srcs[4])
            nc.sync.dma_start(out=x_sbs[6][:], in_=srcs[6])
            nc.sync.dma_start(out=xr_sb[96:128, :], in_=xr[96:128, :])
            nc.sync.dma_start(out=x_sbs[8][:], in_=srcs[8])
            nc.sync.dma_start(out=x_sbs[10][:], in_=srcs[10])
            nc.sync.dma_start(out=x_sbs[12][:], in_=srcs[12])
            nc.sync.dma_start(out=x_sbs[14][:], in_=srcs[14])
            # scalar (ACT) ring
            nc.scalar.dma_start(out=w1m_sb[:, H1:], in_=w1m[:, H1:])
            nc.scalar.dma_start(out=x_sbs[1][:, :hw], in_=srcs[1][:, :hw])
            nc.scalar.dma_start(out=xr_sb[0:32, :], in_=xr[0:32, :])
            nc.scalar.dma_start(out=x_sbs[1][:, hw:], in_=srcs[1][:, hw:])
            nc.scalar.dma_start(out=w1r_sb[:], in_=w1r[:])
            nc.scalar.dma_start(out=b1_sb[:], in_=b1[:])
            nc.scalar.dma_start(out=w2_sb[:], in_=w2[:])
            nc.scalar.dma_start(out=b2_sb[:], in_=b2[:])
            nc.scalar.dma_start(out=x_sbs[3][:], in_=srcs[3])
            nc.scalar.dma_start(out=xr_sb[64:96, :], in_=xr[64:96, :])
            nc.scalar.dma_start(out=x_sbs[5][:], in_=srcs[5])
            nc.scalar.dma_start(out=x_sbs[7][:], in_=srcs[7])
            nc.scalar.dma_start(out=x_sbs[9][:], in_=srcs[9])
            nc.scalar.dma_start(out=x_sbs[11][:], in_=srcs[11])
            nc.scalar.dma_start(out=x_sbs[13][:], in_=srcs[13])
            nc.scalar.dma_start(out=x_sbs[15][:, :hw], in_=srcs[15][:, :hw])
            nc.scalar.dma_start(out=x_sbs[15][:, hw:], in_=srcs[15][:, hw:])

            # PE pre-warm: HAM holds the PE at 1.2 GHz until ~3.4us of
            # sustained activity. A short dummy stream keeps the PE busy
            # from engine-ready until x0 lands; real matmuls then sustain
            # the activity window so K=8/8 engages ~3.4us after start.
            nc.vector.memset(warm_sb[:], 0)
            warm_ps = psh.tile([H1, BT], _f32, tag="ph", name="warm")
            for _ in range(WARM_MM):
                nc.tensor.matmul(
                    warm_ps[:64, :64], warm_sb[:, :64], warm_sb[:, :64],
                    start=True, stop=True,
                )

            # quads of 4 tiles: chunk matmuls pair-shared (LDW,MM,MM), the
            # remainder / fc2 / w1m-reload stationary switches amortize
            # over 4 tiles instead of 2.
            for g in range(N_TILES // 4):
                ts = [4 * g + i for i in range(4)]
                phs = [psh.tile([H1, BT], _f32, tag="ph", name="ph") for _ in range(4)]
                # chunk matmuls, pair-major: (t0,t1) c0..c5, (t2,t3) c0..c5
                for p in range(2):
                    for c in range(FC):
                        for k in (2 * p, 2 * p + 1):
                            nc.tensor.matmul(
                                phs[k][:],
                                w1m_sb[:, c * H1 : (c + 1) * H1],
                                x_sbs[ts[k]][:, c * BT : (c + 1) * BT],
                                start=(c == 0),
                                stop=False,
                            )
                # remainder: one stationary (w1r group g) for all 4 tiles
                for k in range(4):
                    q = ts[k] % 4
                    nc.tensor.matmul(
                        phs[k][:],
                        w1r_sb[32 * g : 32 * g + F_REM, :],
                        xr_sb[32 * g : 32 * g + F_REM, q * BT : (q + 1) * BT],
                        start=False,
                        stop=True,
                        tile_position=(96, 0) if g == 3 else None,
                    )

                # relu(ph + b1) on DVE — ACT stays a pure DMA-issue engine
                hs = [hpool.tile([H1, BT], _f16, tag="h", name="h") for _ in range(4)]
                for k in range(4):
                    nc.vector.tensor_scalar(
                        hs[k][:],
                        phs[k][:],
                        b1_sb[:, 0:1],
                        0.0,
                        mybir.AluOpType.add,
                        mybir.AluOpType.max,
                    )

                # fc2: one w2 load, 4 matmuls
                pos = [pso.tile([H2, BT], _f32, tag="po", name="po") for _ in range(4)]
                for k in range(4):
                    nc.tensor.matmul(
                        pos[k][:], w2_sb[:], hs[k][:], start=True, stop=True,
                    )

                for k in range(4):
                    tt = ts[k]
                    nc.vector.tensor_scalar_add(
                        o_sb[:, tt * BT : (tt + 1) * BT], pos[k][:], b2_sb[:, 0:1]
                    )
                # tapered writeback: the final write after the last tile is
                # only one tile (20 KB)
                flush = {0: [(0, 4)], 1: [(4, 8)], 2: [(8, 12)],
                         3: [(12, 14), (14, 15), (15, 16)]}
                for i, (a, b) in enumerate(flush.get(g, [])):
                    eng = nc.sync if (g + i) % 2 == 1 else nc.scalar
                    eng.dma_start(
                        out=y[:, a * BT : b * BT], in_=o_sb[:, a * BT : b * BT]
                    )

    _split_sync_waits(nc)
    return nc


def _fold_conv_fc1(conv_w: np.ndarray, fc1_w: np.ndarray) -> np.ndarray:
    """Fold the 3x3 valid conv into fc1: W1eff[784, 100] such that
    h = x @ W1eff  ==  fc1( flatten(conv(x)) ).  Accumulated in float64."""
    F = fc1_w.astype(np.float64).T.reshape(26, 26, H1)
    W = np.zeros((28, 28, H1), np.float64)
    cw = conv_w.astype(np.float64)
    for di in range(3):
        for dj in range(3):
            W[di : di + 26, dj : dj + 26, :] += cw[di, dj] * F
    return W.reshape(784, H1).astype(np.float32)


def _make_in_maps(x, conv_w, fc1_w, fc1_b, fc2_w, fc2_b):
    w1eff = _fold_conv_fc1(conv_w, fc1_w)
    w1m = np.ascontiguousarray(
        w1eff[: FC * 128]
        .astype(np.float16)
        .reshape(FC, 128, H1)
        .transpose(1, 0, 2)
        .reshape(128, FC * H1)
    )
    w1r = np.zeros((128, H1), np.float16)
    for g in range(4):
        w1r[32 * g : 32 * g + F_REM] = w1eff[FC * 128 :].astype(np.float16)
    b1 = np.ascontiguousarray(fc1_b.reshape(H1, 1))
    w2 = np.ascontiguousarray(fc2_w.T.astype(np.float16))
    b2 = np.ascontiguousarray(fc2_b.reshape(H2, 1))

    in_maps = []
    for s in range(N_CORES):
        xs = x[s * B_SHARD : (s + 1) * B_SHARD].reshape(N_TILES, BT, 784)
        xm = np.ascontiguousarray(
            xs[:, :, : FC * 128]
            .astype(np.float16)
            .reshape(N_TILES, BT, FC, 128)
            .transpose(0, 3, 2, 1)
        )
        xr_flat = xs.reshape(B_SHARD, 784)[:, FC * 128 :].astype(np.float16)
        xr = np.zeros((128, 4 * BT), np.float16)
        for t in range(N_TILES):
            g, q = t // 4, t % 4
            xr[32 * g : 32 * g + F_REM, q * BT : (q + 1) * BT] = xr_flat[
                t * BT : (t + 1) * BT
            ].T
        in_maps.append(
            {"xm": xm, "xr": xr, "w1m": w1m, "w1r": w1r, "b1": b1, "w2": w2, "b2": b2}
        )
    return in_maps


def _gather(results) -> np.ndarray:
    out = np.empty((B_TOTAL, H2), np.float32)
    for s in range(N_CORES):
        ys = results[s]["y"]  # [H2, B_SHARD]
        out[s * B_SHARD : (s + 1) * B_SHARD] = ys.T
    return out


def kernel_run(inputs: dict, trace: bool = False):
    """Run the kernel; returns (full output (65536,10) f32, BassKernelResults)."""
    x = np.ascontiguousarray(np.asarray(inputs["x"], dtype=np.float32))
    assert x.shape == (B_TOTAL, 784), x.shape
    in_maps = _make_in_maps(
        x,
        np.asarray(inputs["conv_w"], np.float32),
        np.asarray(inputs["fc1_w"], np.float32),
        np.asarray(inputs["fc1_b"], np.float32),
        np.asarray(inputs["fc2_w"], np.float32),
        np.asarray(inputs["fc2_b"], np.float32),
    )
    nc = _build_nc()
    res = run_bass_kernel_spmd(nc, in_maps, core_ids=list(range(N_CORES)), trace=trace)
    return _gather(res.results), res


def kernel(**inputs) -> np.ndarray:
    out, _ = kernel_run(inputs)
    return out



# revision 4
# speedup vs baseline: 1.1027x; 1.1027x over previous
"""Trainium2 Bass kernel for DigitConvolutionalModel (8-core data parallel).

Computation: x(B,784) -> 3x3 valid conv on 28x28 -> flatten(676)
             -> FC(100)+ReLU -> FC(10), B = 65536.

Algebraic restructure (host side, exact): the conv is linear, so conv and
fc1 fold into one 784->100 matrix W1eff (accumulated in float64). The
device kernel is then just two matmul layers per 512-sample tile:
  h = relu(x @ W1eff + b1);  y = h @ fc2_w.T + b2.

Numerics: the matmul datapath runs in fp16 (inputs rounded once on the
host). Measured end-to-end scale-relative absmax error vs the fp32
reference is ~4.5e-4; fp16 streams the PE at 1 col/cycle (fp32 runs at
~1/4 rate) and halves the HBM traffic, which is the kernel's bottleneck.

Per-core layout (B_shard=8192 = 16 tiles x 512):
  x is pre-transposed on the host to feature-major tiles so the matmul
  contraction (features) lands on SBUF partitions with no on-device
  transposes. Features 0..767 form 6 chunks of 128 partitions (full DMA
  port utilization, fully contiguous 0.77 MB loads alternating across
  the two HWDGE rings); the 16 remainder features for all 16 tiles are
  packed into one [128, 2048] tile at 32-aligned partition groups (PE
  row-group granularity) and applied with per-group w1r replicas.
  Outputs accumulate in SBUF and leave in tapered writebacks so only
  one 20 KB write remains after the final tile.
"""

import numpy as np

import concourse.bass as bass
import concourse.mybir as mybir
import concourse.tile as tile
from concourse.bass_utils import run_bass_kernel_spmd
from concourse.vector_clock import ScopedClock

N_CORES = 8
B_TOTAL = 65536
B_SHARD = B_TOTAL // N_CORES  # 8192
BT = 512  # batch tile (one PSUM bank of fp32)
N_TILES = B_SHARD // BT  # 16
FC = 6  # full 128-partition feature chunks (6*128 = 768)
F_REM = 784 - FC * 128  # 16 remainder features
H1 = 100
H2 = 10

_f32 = mybir.dt.float32
_f32r = mybir.dt.float32r
_f16 = mybir.dt.float16


class SplitDrainTileContext(tile.TileContext):
    """TileContext whose tail drain carries at most one sync wait.

    The pinned walrus rejects instructions with >2 sync waits
    ("Too many sync wait commands" in setupSyncWait); the stock tail
    drain accumulates one wait per active proc. Emit one drain per
    wait instead — consecutive drains on the sync engine are
    semantically equivalent to one drain carrying all the waits.
    """

    def _drain_and_barrier(self, tick_clock, wait_clock):
        nc = self.nc
        # Cheap tail: the stock version runs two full EVSEM butterflies
        # (~13us measured). Instead: gpsimd waits on the whole vector
        # clock (all tracked incs have landed), every engine drains its
        # own DGE queues, gpsimd clears the sem ranges, and one
        # sequencer-level sem-only barrier closes the kernel.
        drain_inst = nc.gpsimd.drain()
        wait_clock.add_sem_waits(
            drain_inst.ins, ScopedClock({None: tick_clock.global_clock})
        )
        raw = drain_inst.ins
        si = raw.sync_info
        if si is not None and si.on_wait and len(si.on_wait) > 1:
            waits = list(si.on_wait)
            si.on_wait = waits[:1]
            raw.sync_info = si
            for w in waits[1:]:
                extra = nc.gpsimd.drain()
                extra.ins.sync_info = mybir.SyncInfo(on_wait=[w], on_update=[])
        for eng in (nc.sync, nc.scalar, nc.vector, nc.tensor):
            eng.drain()

        # No tail barrier: gpsimd's global-clock waits above guarantee all
        # tracked sem incs (incl. DMA completions) have landed before the
        # clears, and NRT serializes re-executions on all-engine completion.
        assert self.sems is not None
        popped = nc._tile_sem_poison_stack.pop()
        assert popped is self._sem_poison
        nc.clear_and_free_semaphores(list(self.sems.allocated().values()))


def _split_sync_waits(nc: bass.Bass, limit: int = 1) -> None:
    """Walrus-compat post-pass: the pinned walrus rejects instructions
    carrying more than ~2 sync waits. Hoist excess waits onto NoOp
    instructions inserted just before the offending instruction on the
    same engine — semantically identical (waits run in stream order)."""
    n = 0
    for fn in nc.m.functions:
        for bb in fn.blocks:
            out = []
            changed = False
            for inst in bb.instructions:
                si = inst.sync_info
                if si is not None and si.on_wait and len(si.on_wait) > limit:
                    waits = list(si.on_wait)
                    for i in range(0, len(waits) - limit, limit):
                        nop = mybir.InstNoOp(
                            name=f"swsplit-{n}",
                            ins=[],
                            outs=[],
                            sync_info=mybir.SyncInfo(
                                on_wait=waits[i : i + limit], on_update=[]
                            ),
                        )
                        nop.engine = inst.engine
                        out.append(nop)
                        n += 1
                    si.on_wait = waits[len(waits) - limit :]
                    inst.sync_info = si
                    changed = True
                out.append(inst)
            if changed:
                bb.instructions = out


WARM_MM = 36  # bridges Tensor-ready (~7.6us) to first x bytes (~9.4us)


def _build_nc() -> bass.Bass:
    nc = bass.Bass(monotonic_sem_count=0)
    xm = nc.dram_tensor("xm", [N_TILES, 128, FC, BT], _f16, kind="ExternalInput")
    # remainder features packed 4 tile-groups x 16 features into 128
    # partitions at 32-aligned offsets (PE row-group granularity)
    xr = nc.dram_tensor("xr", [128, 4 * BT], _f16, kind="ExternalInput")
    w1m = nc.dram_tensor("w1m", [128, FC * H1], _f16, kind="ExternalInput")
    # w1r replicated at partition offsets 0/32/64/96
    w1r = nc.dram_tensor("w1r", [128, H1], _f16, kind="ExternalInput")
    b1 = nc.dram_tensor("b1", [H1, 1], _f32, kind="ExternalInput")
    w2 = nc.dram_tensor("w2", [H1, H2], _f16, kind="ExternalInput")
    b2 = nc.dram_tensor("b2", [H2, 1], _f32, kind="ExternalInput")
    y = nc.dram_tensor("y", [H2, N_TILES * BT], _f32, kind="ExternalOutput")

    with SplitDrainTileContext(nc) as tc:
        with (
            tc.tile_pool(name="consts", bufs=1) as cpool,
            tc.tile_pool(name="xp", bufs=8) as xpool,
            tc.tile_pool(name="hp", bufs=4) as hpool,
            tc.tile_pool(name="psh", bufs=4, space="PSUM") as psh,
            tc.tile_pool(name="pso", bufs=4, space="PSUM") as pso,
        ):
            w1m_sb = cpool.tile([128, FC * H1], _f16, tag="w1m")
            xr_sb = cpool.tile([128, 4 * BT], _f16, tag="xr")
            w1r_sb = cpool.tile([128, H1], _f16, tag="w1r")
            b1_sb = cpool.tile([H1, 1], _f32, tag="b1")
            w2_sb = cpool.tile([H1, H2], _f16, tag="w2")
            b2_sb = cpool.tile([H2, 1], _f32, tag="b2")
            # outputs accumulate here; written back in tapered chunks
            o_sb = cpool.tile([H2, N_TILES * BT], _f32, tag="o")
            warm_sb = cpool.tile([128, 64], _f16, tag="warm")

            # w1m chunk 0 loads separately so the first matmul can start
            # as soon as the first half of x0 lands.
            nc.sync.dma_start(out=w1m_sb[:, :H1], in_=w1m[:, :H1])
            nc.sync.dma_start(out=w1m_sb[:, H1:], in_=w1m[:, H1:])

            # PE pre-warm: HAM holds the PE at 1.2 GHz until ~3.4us of
            # sustained activity. A short dummy stream keeps the PE busy
            # from engine-ready until x0 lands; real matmuls then sustain
            # the activity window so K=8/8 engages ~3.4us after start.
            nc.vector.memset(warm_sb[:], 0)
            warm_ps = psh.tile([H1, BT], _f32, tag="ph", name="warm")
            for _ in range(WARM_MM):
                nc.tensor.matmul(
                    warm_ps[:64, :64], warm_sb[:, :64], warm_sb[:, :64],
                    start=True, stop=True,
                )

            # quads of 4 tiles: chunk matmuls pair-shared (LDW,MM,MM), the
            # remainder / fc2 / w1m-reload stationary switches amortize
            # over 4 tiles instead of 2. x DMAs issue just-in-time per
            # tile (the proven 420 GB/s schedule): pool-reuse plus the 8
            # DMAHW sem lanes self-pace the rings ~8 tiles ahead of the
            # PE; upfront bulk issue starves the rings on lane waits.
            hw = FC * BT // 2
            for g in range(N_TILES // 4):
                ts = [4 * g + i for i in range(4)]
                x_sbs = {}
                for t in ts:
                    x_sb = xpool.tile([128, FC * BT], _f16, tag="x", name="x")
                    x_sbs[t] = x_sb
                    dma_eng = nc.sync if t % 2 == 0 else nc.scalar
                    src = xm[t].rearrange("p c b -> p (c b)")
                    if t < 2 or t == N_TILES - 1:
                        # split first/last loads so the PE starts sooner
                        # after t0 and drains sooner after t15
                        dma_eng.dma_start(out=x_sb[:, :hw], in_=src[:, :hw])
                        dma_eng.dma_start(out=x_sb[:, hw:], in_=src[:, hw:])
                    else:
                        dma_eng.dma_start(out=x_sb[:], in_=src)
                    if t == 1:
                        # small consts ride the scalar ring behind x1
                        nc.scalar.dma_start(out=xr_sb[:], in_=xr[:])
                        nc.scalar.dma_start(out=w1r_sb[:], in_=w1r[:])
                        nc.scalar.dma_start(out=b1_sb[:], in_=b1[:])
                        nc.scalar.dma_start(out=w2_sb[:], in_=w2[:])
                        nc.scalar.dma_start(out=b2_sb[:], in_=b2[:])

                phs = [psh.tile([H1, BT], _f32, tag="ph", name="ph") for _ in range(4)]
                # chunk matmuls, pair-major: (t0,t1) c0..c5, (t2,t3) c0..c5
                for p in range(2):
                    for c in range(FC):
                        for k in (2 * p, 2 * p + 1):
                            nc.tensor.matmul(
                                phs[k][:],
                                w1m_sb[:, c * H1 : (c + 1) * H1],
                                x_sbs[ts[k]][:, c * BT : (c + 1) * BT],
                                start=(c == 0),
                                stop=False,
                            )
                # remainder: one stationary (w1r group g) for all 4 tiles
                for k in range(4):
                    q = ts[k] % 4
                    nc.tensor.matmul(
                        phs[k][:],
                        w1r_sb[32 * g : 32 * g + F_REM, :],
                        xr_sb[32 * g : 32 * g + F_REM, q * BT : (q + 1) * BT],
                        start=False,
                        stop=True,
                        tile_position=(96, 0) if g == 3 else None,
                    )

                # relu(ph + b1) on DVE — ACT stays a pure DMA-issue engine
                hs = [hpool.tile([H1, BT], _f16, tag="h", name="h") for _ in range(4)]
                for k in range(4):
                    nc.vector.tensor_scalar(
                        hs[k][:],
                        phs[k][:],
                        b1_sb[:, 0:1],
                        0.0,
                        mybir.AluOpType.add,
                        mybir.AluOpType.max,
                    )

                # fc2: one w2 load, 4 matmuls
                pos = [pso.tile([H2, BT], _f32, tag="po", name="po") for _ in range(4)]
                for k in range(4):
                    nc.tensor.matmul(
                        pos[k][:], w2_sb[:], hs[k][:], start=True, stop=True,
                    )

                for k in range(4):
                    tt = ts[k]
                    nc.vector.tensor_scalar_add(
                        o_sb[:, tt * BT : (tt + 1) * BT], pos[k][:], b2_sb[:, 0:1]
                    )
                # tapered writeback: the final write after the last tile is
                # only one tile (20 KB)
                flush = {0: [(0, 4)], 1: [(4, 8)], 2: [(8, 12)],
                         3: [(12, 14), (14, 15), (15, 16)]}
                for i, (a, b) in enumerate(flush.get(g, [])):
                    eng = nc.sync if (g + i) % 2 == 1 else nc.scalar
                    eng.dma_start(
                        out=y[:, a * BT : b * BT], in_=o_sb[:, a * BT : b * BT]
                    )

    _split_sync_waits(nc)
    return nc


def _fold_conv_fc1(conv_w: np.ndarray, fc1_w: np.ndarray) -> np.ndarray:
    """Fold the 3x3 valid conv into fc1: W1eff[784, 100] such that
    h = x @ W1eff  ==  fc1( flatten(conv(x)) ).  Accumulated in float64."""
    F = fc1_w.astype(np.float64).T.reshape(26, 26, H1)
    W = np.zeros((28, 28, H1), np.float64)
    cw = conv_w.astype(np.float64)
    for di in range(3):
        for dj in range(3):
            W[di : di + 26, dj : dj + 26, :] += cw[di, dj] * F
    return W.reshape(784, H1).astype(np.float32)


def _make_in_maps(x, conv_w, fc1_w, fc1_b, fc2_w, fc2_b):
    w1eff = _fold_conv_fc1(conv_w, fc1_w)
    w1m = np.ascontiguousarray(
        w1eff[: FC * 128]
        .astype(np.float16)
        .reshape(FC, 128, H1)
        .transpose(1, 0, 2)
        .reshape(128, FC * H1)
    )
    w1r = np.zeros((128, H1), np.float16)
    for g in range(4):
        w1r[32 * g : 32 * g + F_REM] = w1eff[FC * 128 :].astype(np.float16)
    b1 = np.ascontiguousarray(fc1_b.reshape(H1, 1))
    w2 = np.ascontiguousarray(fc2_w.T.astype(np.float16))
    b2 = np.ascontiguousarray(fc2_b.reshape(H2, 1))

    in_maps = []
    for s in range(N_CORES):
        xs = x[s * B_SHARD : (s + 1) * B_SHARD].reshape(N_TILES, BT, 784)
        xm = np.ascontiguousarray(
            xs[:, :, : FC * 128]
            .astype(np.float16)
            .reshape(N_TILES, BT, FC, 128)
            .transpose(0, 3, 2, 1)
        )
        xr_flat = xs.reshape(B_SHARD, 784)[:, FC * 128 :].astype(np.float16)
        xr = np.zeros((128, 4 * BT), np.float16)
        for t in range(N_TILES):
            g, q = t // 4, t % 4
            xr[32 * g : 32 * g + F_REM, q * BT : (q + 1) * BT] = xr_flat[
                t * BT : (t + 1) * BT
            ].T
        in_maps.append(
            {"xm": xm, "xr": xr, "w1m": w1m, "w1r": w1r, "b1": b1, "w2": w2, "b2": b2}
        )
    return in_maps


def _gather(results) -> np.ndarray:
    out = np.empty((B_TOTAL, H2), np.float32)
    for s in range(N_CORES):
        ys = results[s]["y"]  # [H2, B_SHARD]
        out[s * B_SHARD : (s + 1) * B_SHARD] = ys.T
    return out


def kernel_run(inputs: dict, trace: bool = False):
    """Run the kernel; returns (full output (65536,10) f32, BassKernelResults)."""
    x = np.ascontiguousarray(np.asarray(inputs["x"], dtype=np.float32))
    assert x.shape == (B_TOTAL, 784), x.shape
    in_maps = _make_in_maps(
        x,
        np.asarray(inputs["conv_w"], np.float32),
        np.asarray(inputs["fc1_w"], np.float32),
        np.asarray(inputs["fc1_b"], np.float32),
        np.asarray(inputs["fc2_w"], np.float32),
        np.asarray(inputs["fc2_b"], np.float32),
    )
    nc = _build_nc()
    res = run_bass_kernel_spmd(nc, in_maps, core_ids=list(range(N_CORES)), trace=trace)
    return _gather(res.results), res


def kernel(**inputs) -> np.ndarray:
    out, _ = kernel_run(inputs)
    return out



# revision 9
# speedup vs baseline: 1.1589x; 1.0510x over previous
"""Trainium2 Bass kernel for DigitConvolutionalModel (8-core data parallel).

Computation: x(B,784) -> 3x3 valid conv on 28x28 -> flatten(676)
             -> FC(100)+ReLU -> FC(10), B = 65536.

Algebraic restructure (host side, exact): the conv is linear, so conv and
fc1 fold into one 784->100 matrix W1eff (accumulated in float64). The
device kernel is then just two matmul layers per 512-sample tile:
  h = relu(x @ W1eff + b1);  y = h @ fc2_w.T + b2.

Numerics: the matmul datapath runs in fp16 (inputs rounded once on the
host). Measured end-to-end scale-relative absmax error vs the fp32
reference is ~4.5e-4; fp16 streams the PE at 1 col/cycle (fp32 runs at
~1/4 rate) and halves the HBM traffic, which is the kernel's bottleneck.

Per-core layout (B_shard=8192 = 16 tiles x 512):
  x is pre-transposed on the host to feature-major tiles so the matmul
  contraction (features) lands on SBUF partitions with no on-device
  transposes. Features 0..767 form 6 chunks of 128 partitions (full DMA
  port utilization, fully contiguous 0.77 MB loads alternating across
  the two HWDGE rings); the 16 remainder features for all 16 tiles are
  packed into one [128, 2048] tile at 32-aligned partition groups (PE
  row-group granularity) and applied with per-group w1r replicas.
  Outputs accumulate in SBUF and leave in tapered writebacks so only
  one 20 KB write remains after the final tile.
"""

import numpy as np

import concourse.bass as bass
import concourse.mybir as mybir
import concourse.tile as tile
from concourse.bass_utils import run_bass_kernel_spmd
from concourse.vector_clock import ScopedClock

N_CORES = 8
B_TOTAL = 65536
B_SHARD = B_TOTAL // N_CORES  # 8192
BT = 512  # batch tile (one PSUM bank of fp32)
N_TILES = B_SHARD // BT  # 16
FC = 6  # full 128-partition feature chunks (6*128 = 768)
F_REM = 784 - FC * 128  # 16 remainder features
H1 = 100
H2 = 10

_f32 = mybir.dt.float32
_f32r = mybir.dt.float32r
_f16 = mybir.dt.float16


class SplitDrainTileContext(tile.TileContext):
    """TileContext whose tail drain carries at most one sync wait.

    The pinned walrus rejects instructions with >2 sync waits
    ("Too many sync wait commands" in setupSyncWait); the stock tail
    drain accumulates one wait per active proc. Emit one drain per
    wait instead — consecutive drains on the sync engine are
    semantically equivalent to one drain carrying all the waits.
    """

    def _drain_and_barrier(self, tick_clock, wait_clock):
        nc = self.nc
        # Cheap tail: the stock version runs two full EVSEM butterflies
        # (~13us measured). Instead: gpsimd waits on the whole vector
        # clock (all tracked incs have landed), every engine drains its
        # own DGE queues, gpsimd clears the sem ranges, and one
        # sequencer-level sem-only barrier closes the kernel.
        drain_inst = nc.gpsimd.drain()
        wait_clock.add_sem_waits(
            drain_inst.ins, ScopedClock({None: tick_clock.global_clock})
        )
        raw = drain_inst.ins
        si = raw.sync_info
        if si is not None and si.on_wait and len(si.on_wait) > 1:
            waits = list(si.on_wait)
            si.on_wait = waits[:1]
            raw.sync_info = si
            for w in waits[1:]:
                extra = nc.gpsimd.drain()
                extra.ins.sync_info = mybir.SyncInfo(on_wait=[w], on_update=[])
        for eng in (nc.sync, nc.scalar, nc.vector, nc.tensor):
            eng.drain()

        # No tail barrier: gpsimd's global-clock waits above guarantee all
        # tracked sem incs (incl. DMA completions) have landed before the
        # clears, and NRT serializes re-executions on all-engine completion.
        assert self.sems is not None
        popped = nc._tile_sem_poison_stack.pop()
        assert popped is self._sem_poison
        nc.clear_and_free_semaphores(list(self.sems.allocated().values()))


def _split_sync_waits(nc: bass.Bass, limit: int = 1) -> None:
    """Walrus-compat post-pass: the pinned walrus rejects instructions
    carrying more than ~2 sync waits. Hoist excess waits onto NoOp
    instructions inserted just before the offending instruction on the
    same engine — semantically identical (waits run in stream order)."""
    n = 0
    for fn in nc.m.functions:
        for bb in fn.blocks:
            out = []
            changed = False
            for inst in bb.instructions:
                si = inst.sync_info
                if si is not None and si.on_wait and len(si.on_wait) > limit:
                    waits = list(si.on_wait)
                    for i in range(0, len(waits) - limit, limit):
                        nop = mybir.InstNoOp(
                            name=f"swsplit-{n}",
                            ins=[],
                            outs=[],
                            sync_info=mybir.SyncInfo(
                                on_wait=waits[i : i + limit], on_update=[]
                            ),
                        )
                        nop.engine = inst.engine
                        out.append(nop)
                        n += 1
                    si.on_wait = waits[len(waits) - limit :]
                    inst.sync_info = si
                    changed = True
                out.append(inst)
            if changed:
                bb.instructions = out


WARM_MM = 64  # bridges Tensor-ready (~8.1us) to w1m+x0 landed (~11.5us)

# mA stream (sync ring head): [w1m | x0], f16 columns
MA_W = FC * H1 + FC * BT  # 600 + 3072
# mB stream (scalar ring head): [x1 | xr | cpk], f16 columns
MB_W = FC * BT + 4 * BT + H1 + H2  # 3072 + 2048 + 110


def _build_nc() -> bass.Bass:
    nc = bass.Bass(monotonic_sem_count=0)
    # tiles 2..15 feature-major (see _make_in_maps)
    xm = nc.dram_tensor("xm", [N_TILES - 2, 128, FC, BT], _f16, kind="ExternalInput")
    # packed head streams: one large contiguous DMA each instead of many
    # small ones -- the ring ramp is overhead-dominated otherwise
    mA = nc.dram_tensor("mA", [128, MA_W], _f16, kind="ExternalInput")
    mB = nc.dram_tensor("mB", [128, MB_W], _f16, kind="ExternalInput")
    # packed fp32 consts: col 0 b1 (rows 0..99), col 1 b2 (rows 0..9)
    cb = nc.dram_tensor("cb", [128, 2], _f32, kind="ExternalInput")
    y = nc.dram_tensor("y", [H2, N_TILES * BT], _f32, kind="ExternalOutput")

    with SplitDrainTileContext(nc) as tc:
        with (
            tc.tile_pool(name="consts", bufs=1) as cpool,
            tc.tile_pool(name="xp", bufs=8) as xpool,
            tc.tile_pool(name="hp", bufs=4) as hpool,
            tc.tile_pool(name="psh", bufs=4, space="PSUM") as psh,
            tc.tile_pool(name="pso", bufs=4, space="PSUM") as pso,
        ):
            # ring heads: sync = [w1m|x0h0], [x0h1]; scalar = [x1h0],
            # [x1h1], [xr|cpk], [cb]. Two big transfers lead each ring.
            mA_sb = cpool.tile([128, MA_W], _f16, tag="mA")
            cutA = FC * H1 + FC * BT // 2
            nc.sync.dma_start(out=mA_sb[:, :cutA], in_=mA[:, :cutA])
            nc.sync.dma_start(out=mA_sb[:, cutA:], in_=mA[:, cutA:])
            mB_sb = cpool.tile([128, MB_W], _f16, tag="mB")
            hw = FC * BT // 2
            nc.scalar.dma_start(out=mB_sb[:, :hw], in_=mB[:, :hw])
            nc.scalar.dma_start(out=mB_sb[:, hw : FC * BT], in_=mB[:, hw : FC * BT])
            nc.scalar.dma_start(out=mB_sb[:, FC * BT :], in_=mB[:, FC * BT :])
            cb_sb = cpool.tile([128, 2], _f32, tag="cb")
            nc.scalar.dma_start(out=cb_sb[:], in_=cb[:])

            w1m_sb = mA_sb[:, : FC * H1]
            xr_sb = mB_sb[:, FC * BT : FC * BT + 4 * BT]
            cpk = mB_sb[:, FC * BT + 4 * BT :]
            w1r_sb = cpk[:, :H1]
            w2_sb = cpk[:H1, H1 : H1 + H2]
            b1_sb = cb_sb[:H1, 0:1]
            b2_sb = cb_sb[:H2, 1:2]
            x_sbs = {0: mA_sb[:, FC * H1 :], 1: mB_sb[:, : FC * BT]}

            # outputs accumulate here; written back in tapered chunks
            o_sb = cpool.tile([H2, N_TILES * BT], _f32, tag="o")
            warm_sb = cpool.tile([128, 64], _f16, tag="warm")

            # PE pre-warm: HAM holds the PE at 1.2 GHz until ~3.4us of
            # sustained activity; it re-throttles after a >3.4us idle
            # gap. The dummy stream covers engine-ready until w1m+x0h0
            # land; intermittent early matmuls then keep the window hot.
            nc.vector.memset(warm_sb[:], 0)
            warm_ps = psh.tile([H1, BT], _f32, tag="ph", name="warm")
            for _ in range(WARM_MM):
                nc.tensor.matmul(
                    warm_ps[:64, :64], warm_sb[:, :64], warm_sb[:, :64],
                    start=True, stop=True,
                )

            def chunk_mm(ph, t, c, start):
                nc.tensor.matmul(
                    ph[:],
                    w1m_sb[:, c * H1 : (c + 1) * H1],
                    x_sbs[t][:, c * BT : (c + 1) * BT],
                    start=start,
                    stop=False,
                )

            def rem_mm(ph, t):
                g, q = t // 4, t % 4
                nc.tensor.matmul(
                    ph[:],
                    w1r_sb[32 * g : 32 * g + F_REM, :],
                    xr_sb[32 * g : 32 * g + F_REM, q * BT : (q + 1) * BT],
                    start=False,
                    stop=True,
                    tile_position=(96, 0) if g == 3 else None,
                )

            def relu(h, ph):
                nc.vector.tensor_scalar(
                    h, ph, b1_sb[:, 0:1], 0.0,
                    mybir.AluOpType.add, mybir.AluOpType.max,
                )

            def issue_x(t, split=False):
                x_sb = xpool.tile([128, FC * BT], _f16, tag="x", name="x")
                x_sbs[t] = x_sb
                eng = nc.sync if t % 2 == 0 else nc.scalar
                src = xm[t - 2].rearrange("p c b -> p (c b)")
                if split:
                    # halves on opposite rings so both rings finish
                    # together and the PE can drain h0 while h1 lands
                    nc.scalar.dma_start(out=x_sb[:, :hw], in_=src[:, :hw])
                    nc.sync.dma_start(out=x_sb[:, hw:], in_=src[:, hw:])
                else:
                    eng.dma_start(out=x_sb[:], in_=src)

            # tiles 0..13 in pairs; x DMAs issue just-in-time (pool-reuse
            # plus the 8 DMAHW sem lanes self-pace the rings ~8 tiles
            # ahead). Tile's scheduler reorders the chunk matmuls
            # tile-major, where per-MM LDWEIGHTS pipelines into the
            # background weight buffer and hides.
            for t in range(14):
                if t >= 2:
                    issue_x(t)
                if t % 2 == 0:
                    continue
                phs = [psh.tile([H1, BT], _f32, tag="ph", name="ph") for _ in range(2)]
                for c in range(FC + 1):
                    for k, tt in enumerate((t - 1, t)):
                        if c < FC:
                            chunk_mm(phs[k], tt, c, start=(c == 0))
                        else:
                            rem_mm(phs[k], tt)
                hs = [hpool.tile([H1, BT], _f16, tag="h", name="h") for _ in range(2)]
                for k in range(2):
                    relu(hs[k][:], phs[k][:])
                pos = [pso.tile([H2, BT], _f32, tag="po", name="po") for _ in range(2)]
                for k in range(2):
                    nc.tensor.matmul(
                        pos[k][:], w2_sb[:], hs[k][:], start=True, stop=True,
                    )
                for k in range(2):
                    tt = t - 1 + k
                    nc.vector.tensor_scalar_add(
                        o_sb[:, tt * BT : (tt + 1) * BT], pos[k][:], b2_sb[:, 0:1]
                    )
                # tapered writeback: big chunks leave mid-kernel
                flush = {7: [(0, 8)], 11: [(8, 12)], 13: [(12, 14)]}
                for i, (a, b) in enumerate(flush.get(t, [])):
                    eng = nc.sync if (t + i) % 2 == 1 else nc.scalar
                    eng.dma_start(
                        out=y[:, a * BT : b * BT], in_=o_sb[:, a * BT : b * BT]
                    )

            # tail: tiles 14, 15 solo so the final dependency chain is
            # one tile deep; t15 in halves so relu/fc2/bias/writeback
            # pipeline with the last half-DMA still in flight
            issue_x(14)
            issue_x(15, split=True)
            ph14 = psh.tile([H1, BT], _f32, tag="ph", name="ph")
            for c in range(FC):
                chunk_mm(ph14, 14, c, start=(c == 0))
            rem_mm(ph14, 14)
            h14 = hpool.tile([H1, BT], _f16, tag="h", name="h")
            relu(h14[:], ph14[:])
            po14 = pso.tile([H2, BT], _f32, tag="po", name="po")
            nc.tensor.matmul(po14[:], w2_sb[:], h14[:], start=True, stop=True)
            nc.vector.tensor_scalar_add(
                o_sb[:, 14 * BT : 15 * BT], po14[:], b2_sb[:, 0:1]
            )
            nc.scalar.dma_start(out=y[:, 14 * BT : 15 * BT],
                                in_=o_sb[:, 14 * BT : 15 * BT])

            ph15 = psh.tile([H1, BT], _f32, tag="ph", name="ph")
            for c in range(FC):
                chunk_mm(ph15, 15, c, start=(c == 0))
            rem_mm(ph15, 15)
            h15 = hpool.tile([H1, BT], _f16, tag="h", name="h")
            po15 = pso.tile([H2, BT], _f32, tag="po", name="po")
            hb = BT // 2
            for j in range(2):
                cols = slice(j * hb, (j + 1) * hb)
                relu(h15[:, cols], ph15[:, cols])
                nc.tensor.matmul(
                    po15[:, cols], w2_sb[:], h15[:, cols], start=True, stop=True,
                )
                nc.vector.tensor_scalar_add(
                    o_sb[:, 15 * BT + j * hb : 15 * BT + (j + 1) * hb],
                    po15[:, cols], b2_sb[:, 0:1],
                )
                eng = nc.sync if j == 0 else nc.scalar
                eng.dma_start(
                    out=y[:, 15 * BT + j * hb : 15 * BT + (j + 1) * hb],
                    in_=o_sb[:, 15 * BT + j * hb : 15 * BT + (j + 1) * hb],
                )

    _split_sync_waits(nc)
    return nc


def _fold_conv_fc1(conv_w: np.ndarray, fc1_w: np.ndarray) -> np.ndarray:
    """Fold the 3x3 valid conv into fc1: W1eff[784, 100] such that
    h = x @ W1eff  ==  fc1( flatten(conv(x)) ).  Accumulated in float64."""
    F = fc1_w.astype(np.float64).T.reshape(26, 26, H1)
    W = np.zeros((28, 28, H1), np.float64)
    cw = conv_w.astype(np.float64)
    for di in range(3):
        for dj in range(3):
            W[di : di + 26, dj : dj + 26, :] += cw[di, dj] * F
    return W.reshape(784, H1).astype(np.float32)


def _make_in_maps(x, conv_w, fc1_w, fc1_b, fc2_w, fc2_b):
    w1eff = _fold_conv_fc1(conv_w, fc1_w)
    w1m = np.ascontiguousarray(
        w1eff[: FC * 128]
        .astype(np.float16)
        .reshape(FC, 128, H1)
        .transpose(1, 0, 2)
        .reshape(128, FC * H1)
    )
    # packed consts: cpk fp16 [128, 110] = [w1r | w2]; cb fp32 [128, 2]
    cpk = np.zeros((128, H1 + H2), np.float16)
    for g in range(4):
        cpk[32 * g : 32 * g + F_REM, :H1] = w1eff[FC * 128 :].astype(np.float16)
    cpk[:H1, H1 : H1 + H2] = fc2_w.T.astype(np.float16)
    cb = np.zeros((128, 2), np.float32)
    cb[:H1, 0] = fc1_b
    cb[:H2, 1] = fc2_b

    in_maps = []
    for s in range(N_CORES):
        xs = x[s * B_SHARD : (s + 1) * B_SHARD].reshape(N_TILES, BT, 784)
        xt = np.ascontiguousarray(
            xs[:, :, : FC * 128]
            .astype(np.float16)
            .reshape(N_TILES, BT, FC, 128)
            .transpose(0, 3, 2, 1)
            .reshape(N_TILES, 128, FC * BT)
        )
        xr_flat = xs.reshape(B_SHARD, 784)[:, FC * 128 :].astype(np.float16)
        xr = np.zeros((128, 4 * BT), np.float16)
        for t in range(N_TILES):
            g, q = t // 4, t % 4
            xr[32 * g : 32 * g + F_REM, q * BT : (q + 1) * BT] = xr_flat[
                t * BT : (t + 1) * BT
            ].T
        # packed head streams: mA = [w1m | x0], mB = [x1 | xr | cpk]
        mA = np.concatenate([w1m, xt[0]], axis=1)
        mB = np.concatenate([xt[1], xr, cpk], axis=1)
        xm = np.ascontiguousarray(xt[2:].reshape(N_TILES - 2, 128, FC, BT))
        in_maps.append({"xm": xm, "mA": mA, "mB": mB, "cb": cb})
    return in_maps


def _gather(results) -> np.ndarray:
    out = np.empty((B_TOTAL, H2), np.float32)
    for s in range(N_CORES):
        ys = results[s]["y"]  # [H2, B_SHARD]
        out[s * B_SHARD : (s + 1) * B_SHARD] = ys.T
    return out


def kernel_run(inputs: dict, trace: bool = False):
    """Run the kernel; returns (full output (65536,10) f32, BassKernelResults)."""
    x = np.ascontiguousarray(np.asarray(inputs["x"], dtype=np.float32))
    assert x.shape == (B_TOTAL, 784), x.shape
    in_maps = _make_in_maps(
        x,
        np.asarray(inputs["conv_w"], np.float32),
        np.asarray(inputs["fc1_w"], np.float32),
        np.asarray(inputs["fc1_b"], np.float32),
        np.asarray(inputs["fc2_w"], np.float32),
        np.asarray(inputs["fc2_b"], np.float32),
    )
    nc = _build_nc()
    res = run_bass_kernel_spmd(nc, in_maps, core_ids=list(range(N_CORES)), trace=trace)
    return _gather(res.results), res


def kernel(**inputs) -> np.ndarray:
    out, _ = kernel_run(inputs)
    return out



# revision 12
# speedup vs baseline: 1.2069x; 1.0414x over previous
"""Trainium2 Bass kernel for DigitConvolutionalModel (8-core data parallel).

Computation: x(B,784) -> 3x3 valid conv on 28x28 -> flatten(676)
             -> FC(100)+ReLU -> FC(10), B = 65536.

Algebraic restructure (host side, exact): the conv is linear, so conv and
fc1 fold into one 784->100 matrix W1eff (accumulated in float64). The
device kernel is then just two matmul layers per 512-sample tile:
  h = relu(x @ W1eff + b1);  y = h @ fc2_w.T + b2.

Numerics: the matmul datapath runs in fp16 (inputs rounded once on the
host). Measured end-to-end scale-relative absmax error vs the fp32
reference is ~4.5e-4; fp16 streams the PE at 1 col/cycle (fp32 runs at
~1/4 rate) and halves the HBM traffic, which is the kernel's bottleneck.

Per-core layout (B_shard=8192 = 16 tiles x 512):
  x is pre-transposed on the host to feature-major tiles so the matmul
  contraction (features) lands on SBUF partitions with no on-device
  transposes. Features 0..767 form 6 chunks of 128 partitions (full DMA
  port utilization, fully contiguous 0.77 MB loads alternating across
  the two HWDGE rings); the 16 remainder features for all 16 tiles are
  packed into one [128, 2048] tile at 32-aligned partition groups (PE
  row-group granularity) and applied with per-group w1r replicas.
  Outputs accumulate in SBUF and leave in tapered writebacks so only
  one 20 KB write remains after the final tile.
"""

import numpy as np

import concourse.bass as bass
import concourse.mybir as mybir
import concourse.tile as tile
from concourse.bass_utils import run_bass_kernel_spmd
from concourse.vector_clock import ScopedClock

N_CORES = 8
B_TOTAL = 65536
B_SHARD = B_TOTAL // N_CORES  # 8192
BT = 512  # batch tile (one PSUM bank of fp32)
N_TILES = B_SHARD // BT  # 16
FC = 6  # full 128-partition feature chunks (6*128 = 768)
F_REM = 784 - FC * 128  # 16 remainder features
H1 = 100
H2 = 10

_f32 = mybir.dt.float32
_f32r = mybir.dt.float32r
_f16 = mybir.dt.float16


class SplitDrainTileContext(tile.TileContext):
    """TileContext whose tail drain carries at most one sync wait.

    The pinned walrus rejects instructions with >2 sync waits
    ("Too many sync wait commands" in setupSyncWait); the stock tail
    drain accumulates one wait per active proc. Emit one drain per
    wait instead — consecutive drains on the sync engine are
    semantically equivalent to one drain carrying all the waits.
    """

    def _drain_and_barrier(self, tick_clock, wait_clock):
        nc = self.nc
        # Cheap tail: the stock version runs two full EVSEM butterflies
        # (~13us measured). Instead: gpsimd waits on the whole vector
        # clock (all tracked incs have landed), every engine drains its
        # own DGE queues, gpsimd clears the sem ranges, and one
        # sequencer-level sem-only barrier closes the kernel.
        drain_inst = nc.gpsimd.drain()
        wait_clock.add_sem_waits(
            drain_inst.ins, ScopedClock({None: tick_clock.global_clock})
        )
        raw = drain_inst.ins
        si = raw.sync_info
        if si is not None and si.on_wait and len(si.on_wait) > 1:
            waits = list(si.on_wait)
            si.on_wait = waits[:1]
            raw.sync_info = si
            for w in waits[1:]:
                extra = nc.gpsimd.drain()
                extra.ins.sync_info = mybir.SyncInfo(on_wait=[w], on_update=[])
        for eng in (nc.sync, nc.scalar, nc.vector, nc.tensor):
            eng.drain()

        # No tail barrier: gpsimd's global-clock waits above guarantee all
        # tracked sem incs (incl. DMA completions) have landed before the
        # clears, and NRT serializes re-executions on all-engine completion.
        assert self.sems is not None
        popped = nc._tile_sem_poison_stack.pop()
        assert popped is self._sem_poison
        nc.clear_and_free_semaphores(list(self.sems.allocated().values()))


def _split_sync_waits(nc: bass.Bass, limit: int = 1) -> None:
    """Walrus-compat post-pass: the pinned walrus rejects instructions
    carrying more than ~2 sync waits. Hoist excess waits onto NoOp
    instructions inserted just before the offending instruction on the
    same engine — semantically identical (waits run in stream order)."""
    n = 0
    for fn in nc.m.functions:
        for bb in fn.blocks:
            out = []
            changed = False
            for inst in bb.instructions:
                si = inst.sync_info
                if si is not None and si.on_wait and len(si.on_wait) > limit:
                    waits = list(si.on_wait)
                    for i in range(0, len(waits) - limit, limit):
                        nop = mybir.InstNoOp(
                            name=f"swsplit-{n}",
                            ins=[],
                            outs=[],
                            sync_info=mybir.SyncInfo(
                                on_wait=waits[i : i + limit], on_update=[]
                            ),
                        )
                        nop.engine = inst.engine
                        out.append(nop)
                        n += 1
                    si.on_wait = waits[len(waits) - limit :]
                    inst.sync_info = si
                    changed = True
                out.append(inst)
            if changed:
                bb.instructions = out


WARM_MM = 64  # bridges Tensor-ready (~8.1us) to w1m+x0 landed (~11.5us)

# mA stream (sync ring head): [w1m | x0], f16 columns
MA_W = FC * H1 + FC * BT  # 600 + 3072
# mB stream (scalar ring head): [x1 | xr | cpk], f16 columns
MB_W = FC * BT + 4 * BT + H1 + H2  # 3072 + 2048 + 110


def _build_nc() -> bass.Bass:
    nc = bass.Bass(monotonic_sem_count=0)
    # tiles 2..15 feature-major (see _make_in_maps)
    xm = nc.dram_tensor("xm", [N_TILES - 2, 128, FC, BT], _f16, kind="ExternalInput")
    # packed head streams: one large contiguous DMA each instead of many
    # small ones -- the ring ramp is overhead-dominated otherwise
    mA = nc.dram_tensor("mA", [128, MA_W], _f16, kind="ExternalInput")
    mB = nc.dram_tensor("mB", [128, MB_W], _f16, kind="ExternalInput")
    # packed fp32 consts: col 0 b1 (rows 0..99), col 1 b2 (rows 0..9)
    cb = nc.dram_tensor("cb", [128, 2], _f32, kind="ExternalInput")
    y = nc.dram_tensor("y", [H2, N_TILES * BT], _f32, kind="ExternalOutput")

    with SplitDrainTileContext(nc) as tc:
        with (
            tc.tile_pool(name="consts", bufs=1) as cpool,
            tc.tile_pool(name="xp", bufs=8) as xpool,
            tc.tile_pool(name="hp", bufs=4) as hpool,
            tc.tile_pool(name="psh", bufs=6, space="PSUM") as psh,
            tc.tile_pool(name="pso", bufs=2, space="PSUM") as pso,
        ):
            # ring heads: sync = [w1m|x0h0], [x0h1]; scalar = [x1h0],
            # [x1h1]. Two big transfers lead each ring; the [xr|cpk]
            # slice of mB and cb issue later, behind x3.
            mA_sb = cpool.tile([128, MA_W], _f16, tag="mA")
            cutA = FC * H1 + FC * BT // 2
            nc.sync.dma_start(out=mA_sb[:, :cutA], in_=mA[:, :cutA])
            nc.sync.dma_start(out=mA_sb[:, cutA:], in_=mA[:, cutA:])
            mB_sb = cpool.tile([128, MB_W], _f16, tag="mB")
            hw = FC * BT // 2
            nc.scalar.dma_start(out=mB_sb[:, :hw], in_=mB[:, :hw])
            nc.scalar.dma_start(out=mB_sb[:, hw : FC * BT], in_=mB[:, hw : FC * BT])
            cb_sb = cpool.tile([128, 2], _f32, tag="cb")

            w1m_sb = mA_sb[:, : FC * H1]
            xr_sb = mB_sb[:, FC * BT : FC * BT + 4 * BT]
            cpk = mB_sb[:, FC * BT + 4 * BT :]
            w1r_sb = cpk[:, :H1]
            w2_sb = cpk[:H1, H1 : H1 + H2]
            b1_sb = cb_sb[:H1, 0:1]
            b2_sb = cb_sb[:H2, 1:2]
            x_sbs = {0: mA_sb[:, FC * H1 :], 1: mB_sb[:, : FC * BT]}

            # outputs accumulate here; written back in tapered chunks
            o_sb = cpool.tile([H2, N_TILES * BT], _f32, tag="o")
            warm_sb = cpool.tile([128, 64], _f16, tag="warm")

            # PE pre-warm: HAM holds the PE at 1.2 GHz until ~3.4us of
            # sustained activity; it re-throttles after a >3.4us idle
            # gap. The dummy stream covers engine-ready until w1m+x0h0
            # land; intermittent early matmuls then keep the window hot.
            nc.vector.memset(warm_sb[:], 0)
            warm_ps = psh.tile([H1, BT], _f32, tag="ph", name="warm")
            for _ in range(WARM_MM):
                nc.tensor.matmul(
                    warm_ps[:64, :64], warm_sb[:, :64], warm_sb[:, :64],
                    start=True, stop=True,
                )

            def chunk_mm(ph, t, c, start):
                nc.tensor.matmul(
                    ph[:],
                    w1m_sb[:, c * H1 : (c + 1) * H1],
                    x_sbs[t][:, c * BT : (c + 1) * BT],
                    start=start,
                    stop=False,
                )

            def rem_mm(ph, t):
                g, q = t // 4, t % 4
                nc.tensor.matmul(
                    ph[:],
                    w1r_sb[32 * g : 32 * g + F_REM, :],
                    xr_sb[32 * g : 32 * g + F_REM, q * BT : (q + 1) * BT],
                    start=False,
                    stop=True,
                    tile_position=(96, 0) if g == 3 else None,
                )

            def relu(h, ph):
                nc.vector.tensor_scalar(
                    h, ph, b1_sb[:, 0:1], 0.0,
                    mybir.AluOpType.add, mybir.AluOpType.max,
                )

            def issue_x(t, eng):
                x_sb = xpool.tile([128, FC * BT], _f16, tag="x", name="x")
                x_sbs[t] = x_sb
                src = xm[t - 2].rearrange("p c b -> p (c b)")
                eng.dma_start(out=x_sb[:], in_=src)

            # Software-pipelined schedule over pairs p = (2p, 2p+1):
            #   PE stream:  A0 A1 [R0] A2 [F0 R1] A3 [F1 R2] ... A7 [F5 R6] [F6 R7] [F7]
            # where A_p = 12 chunk matmuls, R_p = 2 remainder matmuls,
            # F_p = 2 fc2 matmuls. Chunks are DMA-gated; deferring R/F by
            # one pair gives the PE fill work during inter-pair DMA gaps
            # (so HAM never re-throttles) and gives DVE a full pair cycle
            # to produce relu(h) before fc2 consumes it. ph pool (4 bufs)
            # holds exactly the two pairs of live accumulation groups.
            state = {}  # p -> dict(phs, hs, pos)

            def emit_A(p):
                t0, t1 = 2 * p, 2 * p + 1
                phs = [psh.tile([H1, BT], _f32, tag="ph", name="ph") for _ in range(2)]
                state[p] = {"phs": phs}
                for c in range(FC):
                    for k, tt in enumerate((t0, t1)):
                        chunk_mm(phs[k], tt, c, start=(c == 0))

            def emit_R(p):
                st = state[p]
                hs = [hpool.tile([H1, BT], _f16, tag="h", name="h") for _ in range(2)]
                st["hs"] = hs
                for k, tt in enumerate((2 * p, 2 * p + 1)):
                    rem_mm(st["phs"][k], tt)
                for k in range(2):
                    relu(hs[k][:], st["phs"][k][:])

            def emit_F(p, split=False):
                st = state[p]
                pos = [pso.tile([H2, BT], _f32, tag="po", name="po") for _ in range(2)]
                hb = BT // 2
                for k, tt in enumerate((2 * p, 2 * p + 1)):
                    if split and k == 1:
                        continue
                    nc.tensor.matmul(
                        pos[k][:], w2_sb[:], st["hs"][k][:], start=True, stop=True,
                    )
                    nc.vector.tensor_scalar_add(
                        o_sb[:, tt * BT : (tt + 1) * BT], pos[k][:], b2_sb[:, 0:1]
                    )
                if split:
                    # last tile in halves: fc2/bias/writeback pipeline
                    nc.scalar.dma_start(
                        out=y[:, 14 * BT : 15 * BT], in_=o_sb[:, 14 * BT : 15 * BT]
                    )
                    t = 2 * p + 1
                    for j in range(2):
                        cols = slice(j * hb, (j + 1) * hb)
                        nc.tensor.matmul(
                            pos[1][:, cols], w2_sb[:], st["hs"][1][:, cols],
                            start=True, stop=True,
                        )
                        nc.vector.tensor_scalar_add(
                            o_sb[:, t * BT + j * hb : t * BT + (j + 1) * hb],
                            pos[1][:, cols], b2_sb[:, 0:1],
                        )
                        eng = nc.sync if j == 0 else nc.scalar
                        eng.dma_start(
                            out=y[:, t * BT + j * hb : t * BT + (j + 1) * hb],
                            in_=o_sb[:, t * BT + j * hb : t * BT + (j + 1) * hb],
                        )
                # tapered writeback: big chunks leave mid-kernel
                flush = {3: [(0, 4)], 5: [(4, 8)], 6: [(8, 12), (12, 14)]}
                for i, (a, b) in enumerate(flush.get(p, [])):
                    eng = nc.sync if (p + i) % 2 == 1 else nc.scalar
                    eng.dma_start(
                        out=y[:, a * BT : b * BT], in_=o_sb[:, a * BT : b * BT]
                    )

            NP = N_TILES // 2  # 8 pairs
            # x issue schedule: even tiles on sync behind mA, odd on
            # scalar; the [xr|cpk] slice of mB issues after x3 so pair
            # (2,3) data leads it; x15 halves split across both rings so
            # they finish together.
            for p in range(NP):
                t0, t1 = 2 * p, 2 * p + 1
                if p >= 1:
                    if t0 == 14:
                        issue_x(14, nc.sync)
                        x_sb = xpool.tile([128, FC * BT], _f16, tag="x", name="x")
                        x_sbs[15] = x_sb
                        src15 = xm[13].rearrange("p c b -> p (c b)")
                        nc.scalar.dma_start(out=x_sb[:, :hw], in_=src15[:, :hw])
                        nc.sync.dma_start(out=x_sb[:, hw:], in_=src15[:, hw:])
                    else:
                        issue_x(t0, nc.sync)
                        issue_x(t1, nc.scalar)
                    if t1 == 3:
                        nc.scalar.dma_start(
                            out=mB_sb[:, FC * BT :], in_=mB[:, FC * BT :]
                        )
                        nc.scalar.dma_start(out=cb_sb[:], in_=cb[:])
                emit_A(p)
                if p >= 1:
                    if p >= 2:
                        emit_F(p - 2)
                    emit_R(p - 1)
            emit_R(NP - 1)
            emit_F(NP - 2)
            emit_F(NP - 1, split=True)

    _split_sync_waits(nc)
    return nc


def _fold_conv_fc1(conv_w: np.ndarray, fc1_w: np.ndarray) -> np.ndarray:
    """Fold the 3x3 valid conv into fc1: W1eff[784, 100] such that
    h = x @ W1eff  ==  fc1( flatten(conv(x)) ).  Accumulated in float64."""
    F = fc1_w.astype(np.float64).T.reshape(26, 26, H1)
    W = np.zeros((28, 28, H1), np.float64)
    cw = conv_w.astype(np.float64)
    for di in range(3):
        for dj in range(3):
            W[di : di + 26, dj : dj + 26, :] += cw[di, dj] * F
    return W.reshape(784, H1).astype(np.float32)


def _make_in_maps(x, conv_w, fc1_w, fc1_b, fc2_w, fc2_b):
    w1eff = _fold_conv_fc1(conv_w, fc1_w)
    w1m = np.ascontiguousarray(
        w1eff[: FC * 128]
        .astype(np.float16)
        .reshape(FC, 128, H1)
        .transpose(1, 0, 2)
        .reshape(128, FC * H1)
    )
    # packed consts: cpk fp16 [128, 110] = [w1r | w2]; cb fp32 [128, 2]
    cpk = np.zeros((128, H1 + H2), np.float16)
    for g in range(4):
        cpk[32 * g : 32 * g + F_REM, :H1] = w1eff[FC * 128 :].astype(np.float16)
    cpk[:H1, H1 : H1 + H2] = fc2_w.T.astype(np.float16)
    cb = np.zeros((128, 2), np.float32)
    cb[:H1, 0] = fc1_b
    cb[:H2, 1] = fc2_b

    in_maps = []
    for s in range(N_CORES):
        xs = x[s * B_SHARD : (s + 1) * B_SHARD].reshape(N_TILES, BT, 784)
        xt = np.ascontiguousarray(
            xs[:, :, : FC * 128]
            .astype(np.float16)
            .reshape(N_TILES, BT, FC, 128)
            .transpose(0, 3, 2, 1)
            .reshape(N_TILES, 128, FC * BT)
        )
        xr_flat = xs.reshape(B_SHARD, 784)[:, FC * 128 :].astype(np.float16)
        xr = np.zeros((128, 4 * BT), np.float16)
        for t in range(N_TILES):
            g, q = t // 4, t % 4
            xr[32 * g : 32 * g + F_REM, q * BT : (q + 1) * BT] = xr_flat[
                t * BT : (t + 1) * BT
            ].T
        # packed head streams: mA = [w1m | x0], mB = [x1 | xr | cpk]
        mA = np.concatenate([w1m, xt[0]], axis=1)
        mB = np.concatenate([xt[1], xr, cpk], axis=1)
        xm = np.ascontiguousarray(xt[2:].reshape(N_TILES - 2, 128, FC, BT))
        in_maps.append({"xm": xm, "mA": mA, "mB": mB, "cb": cb})
    return in_maps


def _gather(results) -> np.ndarray:
    out = np.empty((B_TOTAL, H2), np.float32)
    for s in range(N_CORES):
        ys = results[s]["y"]  # [H2, B_SHARD]
        out[s * B_SHARD : (s + 1) * B_SHARD] = ys.T
    return out


def kernel_run(inputs: dict, trace: bool = False):
    """Run the kernel; returns (full output (65536,10) f32, BassKernelResults)."""
    x = np.ascontiguousarray(np.asarray(inputs["x"], dtype=np.float32))
    assert x.shape == (B_TOTAL, 784), x.shape
    in_maps = _make_in_maps(
        x,
        np.asarray(inputs["conv_w"], np.float32),
        np.asarray(inputs["fc1_w"], np.float32),
        np.asarray(inputs["fc1_b"], np.float32),
        np.asarray(inputs["fc2_w"], np.float32),
        np.asarray(inputs["fc2_b"], np.float32),
    )
    nc = _build_nc()
    res = run_bass_kernel_spmd(nc, in_maps, core_ids=list(range(N_CORES)), trace=trace)
    return _gather(res.results), res


def kernel(**inputs) -> np.ndarray:
    out, _ = kernel_run(inputs)
    return out

